# revision 3
# baseline (speedup 1.0000x reference)
"""Enframe (overlapping-frame unfold) kernel for Trainium2.

Math: out[b, c*FL + k, t] = x[b, c, t*HOP + k]  with FL=2048, HOP=512,
T = (S - FL)//HOP + 1 = 934.

Decomposition (k = 512*q + 128*i + p, q,i in [0,4), p in [0,128)):
    out[b, c*FL + 512q + 128i + p, t] = X[t+q, 128i+p]
where X[j, r] = x[b, c, j*512 + r] (j < 937). Per (b, c) this is one
937x512 -> 512x937 transpose; each of the 16 output row-blocks is a
contiguous column-slice XT[128i:128(i+1), q:q+934] written densely.

Schedule per core (one batch element per NeuronCore, 8-way data parallel):
  - HBM is the budget (~19.1 MB at ~390 GB/s aggregate over the three DMA
    dispatch rings). The kernel keeps every ring's FIFO non-empty from the
    first load to the last store so the HBM pipe never idles:
      * channel-0 load pieces are enqueued first, spread across all three
        rings (sync + scalar HWDGE, gpsimd SWDGE) so c0 lands ~5 us after
        the preamble; channel-1 pieces queue right behind them.
      * transposes (TensorE via identity matmul, PSUM) start as soon as c0
        is resident; DVE copies assemble each 128-row output block in SBUF.
      * each block's 4 dense ~478 KB store DMAs are assigned to rings by a
        greedy byte-balance so stores drain behind the remaining c1 loads
        and all three rings run dry together at the very end.
  - The NEFF's fixed ~6.3 us full-semaphore-file clear epilogue + barriers
    and the ~1.8 us framework preamble are invariant; everything between is
    paced by HBM.
"""

import numpy as np

import concourse.mybir as mybir
import concourse.tile as tile
from concourse import bacc, bass_utils

B, C, S = 8, 2, 480000
FL, HOP = 2048, 512
T = (S - FL) // HOP + 1          # 934 frames
NQ = FL // HOP                   # 4 hop-shifts per frame length
NJ = T + NQ - 1                  # 937 hop-chunks of input actually used
P = 128
NI = HOP // P                    # 4 row-blocks of 128 within a hop
NJC_FULL = NJ // P               # 7 full 128-row chunks
NJ_REM = NJ - NJC_FULL * P       # 41 remainder rows
F32 = mybir.dt.float32

_NC_CACHE = None


def _emit(tc, nc, x, ident_in, out):
    # x: [C, S] f32 (this core's batch element), out: [C*FL, T] f32
    # HWDGE-only: the two hardware rings (SP + Activation) sustain ~205 GB/s
    # each and, unlike SWDGE, need no Q7 descriptor emission and drain at
    # matched rates; measured two-ring aggregate ~410 GB/s beats the mixed
    # three-ring ~370 GB/s (SWDGE packet round-robin starves the HW rings).
    rings = [nc.sync, nc.scalar]
    queued = [0, 0]              # bytes enqueued per ring, for greedy balance

    def dma(dst, src, nbytes, ring=None):
        if ring is None:
            ring = queued.index(min(queued))
        queued[ring] += nbytes
        rings[ring].dma_start(dst, src)

    with tc.tile_pool(name="consts", bufs=1) as consts, \
         tc.tile_pool(name="loads", bufs=1) as loadp, \
         tc.tile_pool(name="xt", bufs=1) as xtp, \
         tc.tile_pool(name="ps", bufs=8, space="PSUM") as psp:
        ident = consts.tile([P, P], F32, name="ident")
        dma(ident[:, :], ident_in[:, :], P * P * 4, ring=0)

        # Load layout: a_all[p, jc*HOP + r] = x[c, (jc*128 + p)*HOP + r]
        # (dense 2 KB rows per partition per jc chunk); a_rem holds the 41
        # leftover hop-chunks. Pieces go round-robin-by-bytes over the
        # rings, all of c0 enqueued before any of c1, so c0 completes in
        # ~1/3 the single-ring time and transposes/stores start early.
        a_alls, a_rems = [], []
        for c in range(C):
            a_alls.append(
                loadp.tile([P, NJC_FULL * HOP], F32, name=f"a{c}", tag=f"a{c}")
            )
            a_rems.append(
                loadp.tile([NJ_REM, HOP], F32, name=f"ar{c}", tag=f"ar{c}")
            )
        for c in range(C):
            xv_full = x[c, 0:NJC_FULL * P * HOP].rearrange(
                "(jc p r) -> p jc r", p=P, r=HOP
            )
            av = a_alls[c][:, :].rearrange("p (jc r) -> p jc r", r=HOP)
            for jc in range(NJC_FULL):
                dma(av[:, jc:jc + 1], xv_full[:, jc:jc + 1], P * HOP * 4)
            xv = x[c, 0:NJ * HOP].rearrange("(j r) -> j r", r=HOP)
            dma(a_rems[c][:, :], xv[NJC_FULL * P:NJ], NJ_REM * HOP * 4)

        # Transpose + store. xt tiles are distinct per (c, i) so no reuse
        # dependencies gate the pipeline; stores enqueue the moment their
        # block's 8 PSUM->SBUF copies land.
        for c in range(C):
            a_all, a_rem = a_alls[c], a_rems[c]
            for i in range(NI):
                xt = xtp.tile([P, NJ], F32, name=f"xt{c}{i}", tag=f"xt{c}{i}")
                for jc in range(NJC_FULL + 1):
                    if jc < NJC_FULL:
                        j0, nj = jc * P, P
                        src = a_all[:, jc * HOP + i * P: jc * HOP + (i + 1) * P]
                    else:
                        j0, nj = NJC_FULL * P, NJ_REM
                        src = a_rem[:nj, i * P:(i + 1) * P]
                    pt = psp.tile([P, P], F32, name="pt", tag="pt")
                    nc.tensor.transpose(pt[:, :nj], src, ident[:nj, :nj])
                    nc.vector.tensor_copy(xt[:, j0:j0 + nj], pt[:, :nj])
                for q in range(NQ):
                    base = c * FL + q * HOP + i * P
                    dma(out[base:base + P, :], xt[:, q:q + T], P * T * 4)


def _build():
    nc = bacc.Bacc(
        "TRN2",
        target_bir_lowering=False,
        debug=False,
        enable_asserts=False,
        num_devices=B,
    )
    x = nc.dram_tensor("x", [C, S], F32, kind="ExternalInput").ap()
    ident_in = nc.dram_tensor("ident", [P, P], F32, kind="ExternalInput").ap()
    out = nc.dram_tensor("out", [C * FL, T], F32, kind="ExternalOutput").ap()
    with tile.TileContext(nc) as tc:
        _emit(tc, nc, x, ident_in, out)
    nc.compile()
    return nc


def _get_nc():
    global _NC_CACHE
    if _NC_CACHE is None:
        _NC_CACHE = _build()
    return _NC_CACHE


def make_in_maps(x):
    ident = np.eye(P, dtype=np.float32)
    return [
        {"x": np.ascontiguousarray(x[b]), "ident": ident} for b in range(B)
    ]


def kernel(**inputs):
    x = np.ascontiguousarray(np.asarray(inputs["x"]), dtype=np.float32)
    assert x.shape == (B, C, S), x.shape
    nc = _get_nc()
    res = bass_utils.run_bass_kernel_spmd(
        nc, make_in_maps(x), core_ids=list(range(B))
    )
    return np.stack([r["out"] for r in res.results], axis=0)


# revision 4
# speedup vs baseline: 1.0585x; 1.0585x over previous
"""Enframe (overlapping-frame unfold) kernel for Trainium2.

Math: out[b, c*FL + k, t] = x[b, c, t*HOP + k]  with FL=2048, HOP=512,
T = (S - FL)//HOP + 1 = 934.

Decomposition (k = 512*q + 128*i + p, q,i in [0,4), p in [0,128)):
    out[b, c*FL + 512q + 128i + p, t] = X[t+q, 128i+p]
where X[j, r] = x[b, c, j*512 + r] (j < 937). Per (b, c) this is one
937x512 -> 512x937 transpose; each of the 16 output row-blocks is a
contiguous column-slice XT[128i:128(i+1), q:q+934] written densely.

Schedule per core (one batch element per NeuronCore, 8-way data parallel):
  - HBM is the budget (~19.1 MB at ~390 GB/s aggregate over the three DMA
    dispatch rings). The kernel keeps every ring's FIFO non-empty from the
    first load to the last store so the HBM pipe never idles:
      * channel-0 load pieces are enqueued first, spread across all three
        rings (sync + scalar HWDGE, gpsimd SWDGE) so c0 lands ~5 us after
        the preamble; channel-1 pieces queue right behind them.
      * transposes (TensorE via identity matmul, PSUM) start as soon as c0
        is resident; DVE copies assemble each 128-row output block in SBUF.
      * each block's 4 dense ~478 KB store DMAs are assigned to rings by a
        greedy byte-balance so stores drain behind the remaining c1 loads
        and all three rings run dry together at the very end.
  - The NEFF's fixed ~6.3 us full-semaphore-file clear epilogue + barriers
    and the ~1.8 us framework preamble are invariant; everything between is
    paced by HBM.
"""

import numpy as np

import concourse.mybir as mybir
import concourse.tile as tile
from concourse import bacc, bass_utils

B, C, S = 8, 2, 480000
FL, HOP = 2048, 512
T = (S - FL) // HOP + 1          # 934 frames
NQ = FL // HOP                   # 4 hop-shifts per frame length
NJ = T + NQ - 1                  # 937 hop-chunks of input actually used
P = 128
NI = HOP // P                    # 4 row-blocks of 128 within a hop
NJC_FULL = NJ // P               # 7 full 128-row chunks
NJ_REM = NJ - NJC_FULL * P       # 41 remainder rows
F32 = mybir.dt.float32

_NC_CACHE = None


def _emit(tc, nc, x, ident_in, out):
    # x: [C, S] f32 (this core's batch element), out: [C*FL, T] f32
    # Ring split by descriptor size: the 2 KB-per-descriptor loads ride the
    # gpsimd SWDGE ring (Q7 packs up to 64 descriptors per packet, so small
    # descriptors still stream near line rate; HWDGE runs them ~30% slower),
    # while the 3736 B-per-descriptor stores alternate over the two HWDGE
    # rings (SP + Activation), which need no Q7 emission. Channel-0 load
    # pieces go first, split fine so transposes chase the load stream and
    # the first stores issue right after c0 lands; channel-1 queues behind.
    sy, sc, gp = nc.sync, nc.scalar, nc.gpsimd
    store_rr = [0]

    def store_dma(dst, src):
        eng = (sy, sc)[store_rr[0] & 1]
        store_rr[0] += 1
        eng.dma_start(dst, src)

    with tc.tile_pool(name="consts", bufs=1) as consts, \
         tc.tile_pool(name="loads", bufs=1) as loadp, \
         tc.tile_pool(name="xt", bufs=1) as xtp, \
         tc.tile_pool(name="ps", bufs=8, space="PSUM") as psp:
        ident = consts.tile([P, P], F32, name="ident")
        sy.dma_start(ident[:, :], ident_in[:, :])

        # Load layout: a_all[p, jc*HOP + r] = x[c, (jc*128 + p)*HOP + r]
        # (dense 2 KB rows per partition per jc chunk); a_rem holds the 41
        # leftover hop-chunks.
        a_alls, a_rems = [], []
        for c in range(C):
            a_alls.append(
                loadp.tile([P, NJC_FULL * HOP], F32, name=f"a{c}", tag=f"a{c}")
            )
            a_rems.append(
                loadp.tile([NJ_REM, HOP], F32, name=f"ar{c}", tag=f"ar{c}")
            )
        for c in range(C):
            xv_full = x[c, 0:NJC_FULL * P * HOP].rearrange(
                "(jc p r) -> p jc r", p=P, r=HOP
            )
            av = a_alls[c][:, :].rearrange("p (jc r) -> p jc r", r=HOP)
            # c0 in four jc-ordered pieces so transposes pipeline behind the
            # load stream; c1 as two bigger pieces (fewer Q7 emissions).
            splits = ((0, 2), (2, 4), (4, 6), (6, 7)) if c == 0 else \
                     ((0, 4), (4, 7))
            for j0, j1 in splits:
                gp.dma_start(av[:, j0:j1], xv_full[:, j0:j1])
            xv = x[c, 0:NJ * HOP].rearrange("(j r) -> j r", r=HOP)
            gp.dma_start(a_rems[c][:, :], xv[NJC_FULL * P:NJ])

        # Transpose + store. xt tiles are distinct per (c, i) so no reuse
        # dependencies gate the pipeline; each block's 4 dense ~478 KB
        # stores enqueue the moment its 8 PSUM->SBUF copies land.
        for c in range(C):
            a_all, a_rem = a_alls[c], a_rems[c]
            for i in range(NI):
                xt = xtp.tile([P, NJ], F32, name=f"xt{c}{i}", tag=f"xt{c}{i}")
                for jc in range(NJC_FULL + 1):
                    if jc < NJC_FULL:
                        j0, nj = jc * P, P
                        src = a_all[:, jc * HOP + i * P: jc * HOP + (i + 1) * P]
                    else:
                        j0, nj = NJC_FULL * P, NJ_REM
                        src = a_rem[:nj, i * P:(i + 1) * P]
                    pt = psp.tile([P, P], F32, name="pt", tag="pt")
                    nc.tensor.transpose(pt[:, :nj], src, ident[:nj, :nj])
                    nc.vector.tensor_copy(xt[:, j0:j0 + nj], pt[:, :nj])
                for q in range(NQ):
                    base = c * FL + q * HOP + i * P
                    store_dma(out[base:base + P, :], xt[:, q:q + T])


def _build():
    nc = bacc.Bacc(
        "TRN2",
        target_bir_lowering=False,
        debug=False,
        enable_asserts=False,
        num_devices=B,
    )
    x = nc.dram_tensor("x", [C, S], F32, kind="ExternalInput").ap()
    ident_in = nc.dram_tensor("ident", [P, P], F32, kind="ExternalInput").ap()
    out = nc.dram_tensor("out", [C * FL, T], F32, kind="ExternalOutput").ap()
    with tile.TileContext(nc) as tc:
        _emit(tc, nc, x, ident_in, out)
    nc.compile()
    return nc


def _get_nc():
    global _NC_CACHE
    if _NC_CACHE is None:
        _NC_CACHE = _build()
    return _NC_CACHE


def make_in_maps(x):
    ident = np.eye(P, dtype=np.float32)
    return [
        {"x": np.ascontiguousarray(x[b]), "ident": ident} for b in range(B)
    ]


def kernel(**inputs):
    x = np.ascontiguousarray(np.asarray(inputs["x"]), dtype=np.float32)
    assert x.shape == (B, C, S), x.shape
    nc = _get_nc()
    res = bass_utils.run_bass_kernel_spmd(
        nc, make_in_maps(x), core_ids=list(range(B))
    )
    return np.stack([r["out"] for r in res.results], axis=0)


# revision 6
# speedup vs baseline: 1.1089x; 1.0476x over previous
"""Enframe (overlapping-frame unfold) kernel for Trainium2.

Math: out[b, c*FL + k, t] = x[b, c, t*HOP + k]  with FL=2048, HOP=512,
T = (S - FL)//HOP + 1 = 934.

Decomposition (k = 512*q + 128*i + p, q,i in [0,4), p in [0,128)):
    out[b, c*FL + 512q + 128i + p, t] = X[t+q, 128i+p]
where X[j, r] = x[b, c, j*512 + r] (j < 937). Per (b, c) this is one
937x512 -> 512x937 transpose; each of the 16 output row-blocks is a
contiguous column-slice XT[128i:128(i+1), q:q+934] written densely.

Schedule per core (one batch element per NeuronCore, 8-way data parallel):
  - HBM is the budget (~19.1 MB at ~390 GB/s aggregate over the three DMA
    dispatch rings). The kernel keeps every ring's FIFO non-empty from the
    first load to the last store so the HBM pipe never idles:
      * channel-0 load pieces are enqueued first, spread across all three
        rings (sync + scalar HWDGE, gpsimd SWDGE) so c0 lands ~5 us after
        the preamble; channel-1 pieces queue right behind them.
      * transposes (TensorE via identity matmul, PSUM) start as soon as c0
        is resident; DVE copies assemble each 128-row output block in SBUF.
      * each block's 4 dense ~478 KB store DMAs are assigned to rings by a
        greedy byte-balance so stores drain behind the remaining c1 loads
        and all three rings run dry together at the very end.
  - The NEFF's fixed ~6.3 us full-semaphore-file clear epilogue + barriers
    and the ~1.8 us framework preamble are invariant; everything between is
    paced by HBM.
"""

import numpy as np

import concourse.mybir as mybir
import concourse.tile as tile
from concourse import bacc, bass_utils

B, C, S = 8, 2, 480000
FL, HOP = 2048, 512
T = (S - FL) // HOP + 1          # 934 frames
NQ = FL // HOP                   # 4 hop-shifts per frame length
NJ = T + NQ - 1                  # 937 hop-chunks of input actually used
P = 128
NI = HOP // P                    # 4 row-blocks of 128 within a hop
NJC_FULL = NJ // P               # 7 full 128-row chunks
NJ_REM = NJ - NJC_FULL * P       # 41 remainder rows
F32 = mybir.dt.float32

_NC_CACHE = None


def _emit(tc, nc, x, ident_in, out):
    # x: [C, S] f32 (this core's batch element), out: [C*FL, T] f32
    # Ring split by descriptor size: the 2 KB-per-descriptor loads ride the
    # gpsimd SWDGE ring (Q7 packs up to 64 descriptors per packet, so small
    # descriptors still stream near line rate; HWDGE runs them ~30% slower),
    # while the 3736 B-per-descriptor stores alternate over the two HWDGE
    # rings (SP + Activation), which need no Q7 emission. Channel-0 load
    # pieces go first, split fine so transposes chase the load stream and
    # the first stores issue right after c0 lands; channel-1 queues behind.
    sy, sc, gp = nc.sync, nc.scalar, nc.gpsimd
    store_rr = [0]

    def store_dma(dst, src):
        eng = (sy, sc)[store_rr[0] & 1]
        store_rr[0] += 1
        eng.dma_start(dst, src)

    with tc.tile_pool(name="consts", bufs=1) as consts, \
         tc.tile_pool(name="loads", bufs=1) as loadp, \
         tc.tile_pool(name="xt", bufs=1) as xtp, \
         tc.tile_pool(name="ps", bufs=8, space="PSUM") as psp:
        ident = consts.tile([P, P], F32, name="ident")
        sy.dma_start(ident[:, :], ident_in[:, :])

        # Load layout: a_all[p, jc*HOP + r] = x[c, (jc*128 + p)*HOP + r]
        # (dense 2 KB rows per partition per jc chunk); a_rem holds the 41
        # leftover hop-chunks.
        a_alls, a_rems = [], []
        for c in range(C):
            a_alls.append(
                loadp.tile([P, NJC_FULL * HOP], F32, name=f"a{c}", tag=f"a{c}")
            )
            a_rems.append(
                loadp.tile([NJ_REM, HOP], F32, name=f"ar{c}", tag=f"ar{c}")
            )
        for c in range(C):
            xv_full = x[c, 0:NJC_FULL * P * HOP].rearrange(
                "(jc p r) -> p jc r", p=P, r=HOP
            )
            av = a_alls[c][:, :].rearrange("p (jc r) -> p jc r", r=HOP)
            xv = x[c, 0:NJ * HOP].rearrange("(j r) -> j r", r=HOP)
            # The 41-row remainder rides first (it lands on few engines and
            # would otherwise queue behind everything on the slowest one).
            gp.dma_start(a_rems[c][:, :], xv[NJC_FULL * P:NJ])
            if c == 0:
                # c0 split across all three rings, weighted by ring rate
                # (SWDGE ~2.3x a HWDGE ring for these 2 KB descriptors), so
                # c0 is resident ~5 us after the first byte.
                for j0, j1 in ((0, 2), (2, 4)):
                    gp.dma_start(av[:, j0:j1], xv_full[:, j0:j1])
                sy.dma_start(av[:, 4:5], xv_full[:, 4:5])
                sc.dma_start(av[:, 5:7], xv_full[:, 5:7])
            else:
                # c1 streams behind c0 on the SWDGE ring only, keeping the
                # HWDGE rings clear for c0's stores.
                for j0, j1 in ((0, 2), (2, 4), (4, 7)):
                    gp.dma_start(av[:, j0:j1], xv_full[:, j0:j1])

        # Transpose + store. xt tiles are distinct per (c, i) so no reuse
        # dependencies gate the pipeline; each block's 4 dense ~478 KB
        # stores enqueue the moment its 8 PSUM->SBUF copies land.
        for c in range(C):
            a_all, a_rem = a_alls[c], a_rems[c]
            for i in range(NI):
                xt = xtp.tile([P, NJ], F32, name=f"xt{c}{i}", tag=f"xt{c}{i}")
                for jc in (NJC_FULL, *range(NJC_FULL)):
                    if jc < NJC_FULL:
                        j0, nj = jc * P, P
                        src = a_all[:, jc * HOP + i * P: jc * HOP + (i + 1) * P]
                    else:
                        j0, nj = NJC_FULL * P, NJ_REM
                        src = a_rem[:nj, i * P:(i + 1) * P]
                    pt = psp.tile([P, P], F32, name="pt", tag="pt")
                    nc.tensor.transpose(pt[:, :nj], src, ident[:nj, :nj])
                    nc.vector.tensor_copy(xt[:, j0:j0 + nj], pt[:, :nj])
                for q in range(NQ):
                    base = c * FL + q * HOP + i * P
                    store_dma(out[base:base + P, :], xt[:, q:q + T])


def _build():
    nc = bacc.Bacc(
        "TRN2",
        target_bir_lowering=False,
        debug=False,
        enable_asserts=False,
        num_devices=B,
    )
    x = nc.dram_tensor("x", [C, S], F32, kind="ExternalInput").ap()
    ident_in = nc.dram_tensor("ident", [P, P], F32, kind="ExternalInput").ap()
    out = nc.dram_tensor("out", [C * FL, T], F32, kind="ExternalOutput").ap()
    with tile.TileContext(nc) as tc:
        _emit(tc, nc, x, ident_in, out)
    nc.compile()
    return nc


def _get_nc():
    global _NC_CACHE
    if _NC_CACHE is None:
        _NC_CACHE = _build()
    return _NC_CACHE


def make_in_maps(x):
    ident = np.eye(P, dtype=np.float32)
    return [
        {"x": np.ascontiguousarray(x[b]), "ident": ident} for b in range(B)
    ]


def kernel(**inputs):
    x = np.ascontiguousarray(np.asarray(inputs["x"]), dtype=np.float32)
    assert x.shape == (B, C, S), x.shape
    nc = _get_nc()
    res = bass_utils.run_bass_kernel_spmd(
        nc, make_in_maps(x), core_ids=list(range(B))
    )
    return np.stack([r["out"] for r in res.results], axis=0)
